# revision 45
# baseline (speedup 1.0000x reference)
"""Trainium2 Bass kernel for local (windowed causal) self-attention.

Problem: B=2, T=2048, C=1024, 16 heads x 64 dim, local window 256.
Sharding: T-sharding. 8 cores = 2 batches x 4 chunks of 512 tokens.
Each core receives its 512-token chunk plus a 256-token left halo of x
(zero-padded for chunk 0), computes QKV / banded attention / output
projection for its own rows, and writes a disjoint [512, 1024] slice of
the output. No collectives; the host concatenates the 8 slices.

vs the first working version: host pre-transposes x and packs weights
into SBUF-tile-shaped arrays (kills the on-chip transpose phase and
makes every weight DMA one dense block), input DMAs are issued in
first-needed order, K projection runs as 512+256 column chunks, masks
are multiplicative 0/1 applied post-exp on the vector engine, the two
64-row score matmuls of each head-pair draw PSUM from separate pools
(pa/pb) so they dispatch back-to-back and overlap in distinct PE row
groups, and y is written out in bf16 (host upcasts). fp8 DoubleRow
for Q/K was tried and measured SLOWER than bf16 (behind KERNEL_QK_DT).

Self-contained: hardcodes all shapes; no reads of /root/problem/*.
"""

import os

os.environ.setdefault("MYCRO_LOCAL_CACHE", "1")

import numpy as np

# ---------------------------------------------------------------- constants
B, T, C = 2, 2048, 1024
H, D = 16, 64
WIN = 256                      # local attention context
NCORES = 8
CHUNK = 512                    # queries per core
HALO = 256                     # left halo (== WIN)
TQ = CHUNK + HALO              # 768 x rows per core
P = 128

NQT = CHUNK // P               # 4 query tiles per core
NKT = TQ // P                  # 6 key tiles per core

W_SCALE = 64.0                 # fp8 scaling of W_attn Q/K columns

# (kt, qt) pairs whose P^T slab block needs a 0/1 mask multiply.
# j = kt - qt: j==0 -> window edge; j==2 -> causal edge; (1,0) is all-valid
# generically but fully invalid on the boundary chunk (keys < 0), included
# so every core runs an identical instruction stream.
MASK_PAIRS = [(0, 0), (1, 1), (2, 2), (3, 3),
              (1, 0),
              (2, 0), (3, 1), (4, 2), (5, 3)]
NMASK = len(MASK_PAIRS)

# Q/K projection matmul dtype: fp8 DoubleRow ("fp8") or bf16 ("bf16").
# fp8 DoubleRow measured SLOWER than bf16 on this hw (DR matmul runs
# 2N cycles + 13%, i.e. the "2x" never materializes) - default bf16.
QK_DT = os.environ.get("KERNEL_QK_DT", "bf16")
# fp8 for the Q/K *weights only* (stationary side, normal-mode matmul):
# same PE speed, but halves the weight DMA bytes on the critical head
# phase. Scores take ~2x bf16 quantization error - still well in budget.
WQK_FP8 = os.environ.get("KERNEL_WQK_FP8", "0") == "1" and QK_DT == "bf16"
MASK_ENGINE = os.environ.get("KERNEL_MASK_ENGINE", "vector")

_MODS = {}                     # cached compiled Bass modules


def _np_f8():
    import ml_dtypes
    return np.dtype(ml_dtypes.float8_e4m3fn)


def _np_bf16():
    import ml_dtypes
    return np.dtype(ml_dtypes.bfloat16)


# ------------------------------------------------------------- bass builder
def _build_module(zero_bias):
    import concourse.bacc as bacc
    import concourse.mybir as mybir
    import concourse.tile as tile
    from concourse.masks import make_identity
    from contextlib import ExitStack

    F32 = mybir.dt.float32
    BF16 = mybir.dt.bfloat16
    FP8 = mybir.dt.float8e4
    QDT = FP8 if QK_DT == "fp8" else BF16          # x dtype for Q/K path
    WDT = FP8 if (QK_DT == "fp8" or WQK_FP8) else BF16   # Q/K weight dtype
    DR = mybir.MatmulPerfMode.DoubleRow if QK_DT == "fp8" else None

    nc = bacc.Bacc(
        "TRN2",
        target_bir_lowering=False,
        debug=False,
        enable_asserts=False,
        num_devices=NCORES,
    )

    # x^T, halo-padded, pre-transposed on host: [ct, p, t]
    xt8 = nc.dram_tensor("xt8", [C // P, P, TQ], QDT, kind="ExternalInput").ap()
    xtb = nc.dram_tensor("xtb", [C // P, P, TQ], BF16, kind="ExternalInput").ap()
    # Q/K weights, fp8 x64, DoubleRow layout: [jt(16), p, ctp(4), s(2), j(128)]
    if QK_DT == "fp8":
        wqk = nc.dram_tensor("wqk", [16, P, 4, 2, P], WDT,
                             kind="ExternalInput").ap()
    else:
        wqk = nc.dram_tensor("wqk", [16, P, 8, P], WDT,
                             kind="ExternalInput").ap()
    # V weights: [vc(2), ct(8), p, m(512)]
    wv = nc.dram_tensor("wv", [2, C // P, P, 512], BF16,
                        kind="ExternalInput").ap()
    # proj weights: [oc(2), hp(8), p, m(512)]
    wp = nc.dram_tensor("wp", [2, H // 2, P, 512], BF16,
                        kind="ExternalInput").ap()
    ba = nc.dram_tensor("ba", [3 * C], F32, kind="ExternalInput").ap()
    bp = nc.dram_tensor("bp", [C], F32, kind="ExternalInput").ap()
    mk = nc.dram_tensor("mk", [P, NMASK, P], BF16, kind="ExternalInput").ap()
    # y in bf16: halves the writeout DMA on the critical tail; the host
    # upcasts to f32 (values come from f32 PSUM, only one final rounding)
    y = nc.dram_tensor("y", [CHUNK, C], BF16, kind="ExternalOutput").ap()

    Exp = mybir.ActivationFunctionType.Exp
    Ident = mybir.ActivationFunctionType.Identity
    ADD = mybir.AluOpType.add
    MUL = mybir.AluOpType.mult

    # exp scale: 1/sqrt(D) plus fp8 W_SCALE^2 compensation
    escale = (1.0 / np.sqrt(D))
    if QK_DT == "fp8" or WQK_FP8:
        escale /= (W_SCALE * W_SCALE)

    with tile.TileContext(nc) as tc, ExitStack() as ctx:
        const = ctx.enter_context(tc.tile_pool(name="const", bufs=1))
        big = ctx.enter_context(tc.tile_pool(name="big", bufs=1))
        wqpool = ctx.enter_context(tc.tile_pool(name="wqpool", bufs=6))
        wvpool = ctx.enter_context(tc.tile_pool(name="wvpool", bufs=16))
        wppool = ctx.enter_context(tc.tile_pool(name="wppool", bufs=16))
        slabp = ctx.enter_context(tc.tile_pool(name="slabp", bufs=16))
        small = ctx.enter_context(tc.tile_pool(name="small", bufs=12))
        yout = ctx.enter_context(tc.tile_pool(name="yout", bufs=4))
        brow = ctx.enter_context(tc.tile_pool(name="brow", bufs=2))
        # pa: K/Q/V/proj chains + hh0 scores; pb: hh1 scores. Separate
        # pools so a score pair's two PSUM slots free together and the two
        # 64-row matmuls dispatch back-to-back (PE row-group concurrency).
        pa = ctx.enter_context(tc.tile_pool(name="pa", bufs=4, space="PSUM"))
        pb = ctx.enter_context(tc.tile_pool(name="pb", bufs=2, space="PSUM"))
        pc = ctx.enter_context(tc.tile_pool(name="pc", bufs=2, space="PSUM"))

        # ---------------- constants
        identv = const.tile([P, P], BF16)
        make_identity(nc, identv)

        if not zero_bias:
            bqk = const.tile([P, 16], F32)      # b_attn[:2048] as [128, jt]
            with nc.allow_non_contiguous_dma(reason="tiny bias rearrange"):
                nc.sync.dma_start(
                    bqk, ba[: 2 * C].rearrange("(j p) -> p j", p=P))
            bv_row = brow.tile([1, C], F32, tag="brow")
            nc.sync.dma_start(bv_row, ba[None, 2 * C:])
            bv_b = const.tile([P, C], F32)
            nc.gpsimd.partition_broadcast(bv_b, bv_row)
            bp_row = brow.tile([1, C], F32, tag="brow")
            nc.sync.dma_start(bp_row, bp[None, :])
            bp_b = const.tile([P, C], F32)
            nc.gpsimd.partition_broadcast(bp_b, bp_row)

        # PE warm-up: dense dummy matmuls while the first DMAs land, so the
        # HAM clock-gate ramps before real matmuls start (~3.4us window).
        warm = const.tile([P, 512], BF16)
        nc.gpsimd.memset(warm, 0.0)
        for wi in range(14):
            wps = pa.tile([P, 512], F32, tag="pa", name=f"wps{wi}")
            nc.tensor.matmul(wps, warm[:, :P], warm, start=True, stop=True)

        masks = const.tile([P, NMASK, P], BF16)

        # ---------------- QKV holders
        xT8 = big.tile([P, C // P, TQ], QDT, tag="xT8")
        xTb = big.tile([P, C // P, TQ], BF16, tag="xTb")
        QT = big.tile([P, 8, CHUNK], BF16, tag="QT")   # [128j, jt, 512t(own)]
        KT = big.tile([P, 8, TQ], BF16, tag="KT")      # [128j, jt, 768t]
        # V natural + ones columns: [128t, tt, head, D+2]
        VS = big.tile([P, NKT, H, D + 2], BF16, tag="VS")
        ones_h = const.tile([P, NKT * H], F32)
        nc.gpsimd.memset(ones_h, 1.0)
        nc.vector.tensor_copy(
            VS[:, :, :, D], ones_h.rearrange("p (t h) -> p t h", h=H))
        nc.vector.tensor_copy(
            VS[:, :, :, D + 1], ones_h.rearrange("p (t h) -> p t h", h=H))

        def qk_matmuls(ps, jt, t_lo, t_hi):
            """Emit the contraction chain for Q/K feature tile jt over
            x columns [t_lo, t_hi). ps is the PSUM target [P, t_hi-t_lo]."""
            if QK_DT == "fp8":
                wt = wq_tiles[jt]
                for ctp in range(4):
                    nc.tensor.matmul(
                        ps,
                        wt[:, ctp, :, :],
                        xT8[:, 2 * ctp:2 * ctp + 2, t_lo:t_hi],
                        perf_mode=DR,
                        start=(ctp == 0), stop=(ctp == 3))
            else:
                wt = wq_tiles[jt]
                for ct in range(8):
                    nc.tensor.matmul(
                        ps,
                        wt[:, ct, :],
                        xT8[:, ct, t_lo:t_hi],
                        start=(ct == 0), stop=(ct == 7))

        wq_tiles = {}

        def load_wq(jt):
            if QK_DT == "fp8":
                wt = wqpool.tile([P, 4, 2, P], WDT, tag="wq")
            else:
                wt = wqpool.tile([P, 8, P], WDT, tag="wq")
            nc.sync.dma_start(wt, wqk[jt])
            wq_tiles[jt] = wt

        # ---------------- input DMAs, priority order: the Q-part's first
        # tiles go first so the PE can start as soon as possible.
        load_wq(0)
        nc.sync.dma_start(xT8[:, 0, :], xt8[0])
        nc.sync.dma_start(xT8[:, 1, :], xt8[1])
        load_wq(1)
        nc.sync.dma_start(xT8[:, 2, :], xt8[2])
        nc.sync.dma_start(xT8[:, 3, :], xt8[3])
        load_wq(2)
        for ct in range(4, C // P):
            nc.sync.dma_start(xT8[:, ct, :], xt8[ct])
        nc.sync.dma_start(masks, mk)

        # ---------------- Q part
        for jt in range(8):
            if jt + 2 < 8 and (jt + 2) not in wq_tiles:
                load_wq(jt + 2)
            ps = pa.tile([P, CHUNK], F32, tag="pa")
            qk_matmuls(ps, jt, HALO, TQ)
            nc.scalar.activation(
                QT[:, jt, :], ps, Ident, scale=1.0,
                bias=0.0 if zero_bias else bqk[:, jt:jt + 1])

        # x bf16 copy + V weights, needed from the V part onwards
        # (when the Q/K path is already bf16, xT8 doubles as the V-part x)
        if QK_DT == "fp8":
            for ct in range(C // P):
                nc.sync.dma_start(xTb[:, ct, :], xtb[ct])
        else:
            xTb = xT8

        # ---------------- V part (bf16)
        for vc in range(2):
            wts = []
            for ct in range(C // P):
                wt = wvpool.tile([P, 512], BF16, tag="wv")
                nc.sync.dma_start(wt, wv[vc, ct])
                wts.append(wt)
            for tt in range(NKT):
                ps = pa.tile([P, 512], F32, tag="pa")
                for ct in range(C // P):
                    nc.tensor.matmul(
                        ps,
                        xTb[:, ct, tt * P:(tt + 1) * P],
                        wts[ct],
                        start=(ct == 0), stop=(ct == C // P - 1))
                if zero_bias:
                    nc.vector.tensor_copy(
                        VS[:, tt, vc * 8:(vc + 1) * 8, 0:D],
                        ps.rearrange("p (h d) -> p h d", d=D))
                else:
                    nc.vector.tensor_tensor(
                        VS[:, tt, vc * 8:(vc + 1) * 8, 0:D],
                        ps.rearrange("p (h d) -> p h d", d=D),
                        bv_b[:, vc * 512:(vc + 1) * 512]
                            .rearrange("p (h d) -> p h d", d=D),
                        ADD)

        # ---------------- K part + attention head-pairs interleaved
        outT = big.tile([P, 8, CHUNK], BF16, tag="outT")  # [c_pair, hp, t]
        mask_by_kt = {}
        for mi, (kt, qt) in enumerate(MASK_PAIRS):
            mask_by_kt.setdefault(kt, []).append((mi, qt))
        mask_tt = nc.gpsimd.tensor_tensor if MASK_ENGINE == "gpsimd" \
            else nc.vector.tensor_tensor

        def emit_attention_pair(hp):
            pair = [small.tile([P, P], BF16, tag="pair",
                               name=f"pair{hp}_{i}")
                    for i in range(NQT)]
            slabs2 = [[], []]
            for kt in range(NKT):
                qlo = max(0, kt - 2)
                qhi = min(NQT - 1, kt)
                nq = (qhi - qlo + 1) * P
                pss = []
                for hh in range(2):              # row-tiled 64-partition MMs
                    p0 = hh * 64
                    pool, ptag = (pa, "pa") if hh == 0 else (pb, "pb")
                    ps = pool.tile([P, 384], F32, tag=ptag,
                                   name=f"st{hp}_{kt}_{hh}")
                    nc.tensor.matmul(
                        ps[:, :nq],
                        KT[p0:p0 + 64, hp, kt * P:(kt + 1) * P],
                        QT[p0:p0 + 64, hp, qlo * P: qlo * P + nq],
                        start=True, stop=True)
                    pss.append(ps)
                for hh in range(2):
                    ps = pss[hh]
                    slab = slabp.tile([P, 384], BF16, tag="slab",
                                      name=f"slab{hp}_{kt}_{hh}")
                    nc.scalar.activation(slab[:, :nq], ps[:, :nq], Exp,
                                         bias=0.0, scale=float(escale))
                    for mi, qt in mask_by_kt.get(kt, ()):
                        qoff = (qt - qlo) * P
                        mask_tt(slab[:, qoff:qoff + P],
                                slab[:, qoff:qoff + P],
                                masks[:, mi, :], MUL)
                    slabs2[hh].append(slab)

            for hh in range(2):
                h = 2 * hp + hh
                p0 = hh * 64
                slabs = slabs2[hh]
                for qt in range(NQT):
                    pav = pc.tile([P, D + 2], F32, tag="pc")
                    # kt = qt+1 first: it is unmasked, so its slab is ready
                    # earliest; the masked kt=qt / kt=qt+2 slabs get slack.
                    for i, kt in enumerate((qt + 1, qt, qt + 2)):
                        qoff = (qt - max(0, kt - 2)) * P
                        nc.tensor.matmul(
                            pav,
                            slabs[kt][:, qoff:qoff + P],
                            VS[:, kt, h, :],
                            start=(i == 0), stop=(i == 2))
                    rcp = small.tile([P, 1], F32, tag="rcp")
                    nc.vector.reciprocal(rcp, pav[:, D:D + 1])
                    nc.vector.tensor_scalar_mul(
                        pair[qt][:, p0:p0 + 64], pav[:, 0:D], rcp)

            # transpose head-pair outputs into c_in-major layout
            for qt in range(NQT):
                pt = pc.tile([P, P], BF16, tag="pc")
                nc.tensor.transpose(pt, pair[qt], identv)
                nc.vector.tensor_copy(outT[:, hp, qt * P:(qt + 1) * P], pt)

        load_wq(8)
        load_wq(9)
        wp_tiles = {}

        def emit_K(jt):
            # 768 = 512 + 256 column chunks
            for t_lo, t_hi in ((0, 512), (512, 768)):
                ps = pa.tile([P, t_hi - t_lo], F32, tag="pa")
                qk_matmuls(ps, 8 + jt, t_lo, t_hi)
                nc.scalar.activation(
                    KT[:, jt, t_lo:t_hi], ps, Ident,
                    scale=1.0,
                    bias=0.0 if zero_bias else bqk[:, 8 + jt: 9 + jt])

        for jt in range(8):
            if jt + 2 < 8:
                load_wq(10 + jt)
            emit_K(jt)
            emit_attention_pair(jt)
            if jt == 3:                       # prefetch proj weights
                for oc in range(2):
                    for hp in range(8):
                        wt = wppool.tile([P, 512], BF16, tag="wp")
                        nc.sync.dma_start(wt, wp[oc, hp])
                        wp_tiles[(oc, hp)] = wt

        # ---------------- output projection
        for oc in range(2):
            for tb in range(NQT):
                ps = pa.tile([P, 512], F32, tag="pa")
                for hp in range(8):
                    nc.tensor.matmul(
                        ps,
                        outT[:, hp, tb * P:(tb + 1) * P],
                        wp_tiles[(oc, hp)],
                        start=(hp == 0), stop=(hp == 7))
                ysb = yout.tile([P, 512], BF16, tag="ysb")
                if zero_bias:
                    nc.vector.tensor_copy(ysb, ps)
                else:
                    nc.vector.tensor_tensor(
                        ysb, ps, bp_b[:, oc * 512:(oc + 1) * 512], ADD)
                nc.sync.dma_start(
                    y[tb * P:(tb + 1) * P, oc * 512:(oc + 1) * 512], ysb)

    nc.compile()
    return nc


def _get_module(zero_bias):
    if zero_bias not in _MODS:
        _MODS[zero_bias] = _build_module(zero_bias)
    return _MODS[zero_bias]


# ------------------------------------------------------------- host helpers
def _mask_tiles(chunk_start: int) -> np.ndarray:
    """[128, NMASK, 128] multiplicative mask tiles (1 valid / 0 invalid)."""
    out = np.zeros((P, NMASK, P), np.float32)
    kk = np.arange(P)[:, None]
    qq = np.arange(P)[None, :]
    for mi, (kt, qt) in enumerate(MASK_PAIRS):
        key_abs = chunk_start - HALO + kt * P + kk
        q_abs = chunk_start + qt * P + qq
        valid = (key_abs <= q_abs) & (key_abs >= q_abs - WIN) & (key_abs >= 0)
        out[:, mi, :] = np.where(valid, 1.0, 0.0).astype(np.float32)
    return out


def _pack_inputs(x, W_attn, b_attn, W_proj, b_proj):
    """Shared (per-core-independent) packed weight arrays."""
    f8, bf = _np_f8(), _np_bf16()
    W_attn = np.asarray(W_attn, np.float32)
    W_proj = np.asarray(W_proj, np.float32)
    # Q/K weights
    if QK_DT == "fp8":
        wqk = np.empty((16, P, 4, 2, P), f8)
        src = (W_attn[:, :2 * C] * W_SCALE).astype(f8)
        # wqk[jt, p, ctp, s, j] = 64*W[(2ctp+s)*128+p, jt*128+j]
        v = src.reshape(4, 2, P, 16, P)          # [ctp, s, p, jt, j]
        wqk[:] = np.ascontiguousarray(v.transpose(3, 2, 0, 1, 4))
    elif WQK_FP8:
        wqk = np.empty((16, P, 8, P), f8)
        v = (W_attn[:, :2 * C] * W_SCALE).astype(f8).reshape(8, P, 16, P)
        wqk[:] = np.ascontiguousarray(v.transpose(2, 1, 0, 3))
    else:
        wqk = np.empty((16, P, 8, P), bf)
        v = W_attn[:, :2 * C].astype(bf).reshape(8, P, 16, P)
        wqk[:] = np.ascontiguousarray(v.transpose(2, 1, 0, 3))
    # V weights [vc, ct, p, m]
    v = W_attn[:, 2 * C:].astype(bf).reshape(8, P, 2, 512)
    wvp = np.ascontiguousarray(v.transpose(2, 0, 1, 3))
    # proj weights [oc, hp, p, m]
    v = W_proj.astype(bf).reshape(8, P, 2, 512)
    wpp = np.ascontiguousarray(v.transpose(2, 0, 1, 3))
    ba = np.asarray(b_attn, np.float32).copy()
    if QK_DT == "fp8" or WQK_FP8:
        # Q/K carry one factor of W_SCALE each; bias must match
        ba[:2 * C] *= W_SCALE
    bpp = np.ascontiguousarray(b_proj, np.float32)
    return wqk, wvp, wpp, ba, bpp


def _in_maps(x, W_attn, b_attn, W_proj, b_proj):
    f8, bf = _np_f8(), _np_bf16()
    wqk, wvp, wpp, ba, bpp = _pack_inputs(x, W_attn, b_attn, W_proj, b_proj)
    x = np.asarray(x, np.float32)
    maps = []
    for c in range(NCORES):
        b, k = divmod(c, NCORES // B)
        t0 = k * CHUNK
        xhalo = np.zeros((TQ, C), np.float32)
        lo = t0 - HALO
        src_lo = max(0, lo)
        xhalo[src_lo - lo:, :] = x[b, src_lo: t0 + CHUNK]
        xT = np.ascontiguousarray(xhalo.T)            # [C, TQ]
        xt_dt = f8 if QK_DT == "fp8" else bf
        maps.append({
            "xt8": np.ascontiguousarray(
                xT.reshape(C // P, P, TQ).astype(xt_dt)),
            "xtb": np.ascontiguousarray(
                xT.reshape(C // P, P, TQ).astype(bf)),
            "wqk": wqk,
            "wv": wvp,
            "wp": wpp,
            "ba": ba,
            "bp": bpp,
            "mk": _mask_tiles(t0).astype(bf),
        })
    return maps


def _run(inputs, trace=False, trace_kwargs=None):
    from concourse import bass_utils

    zero_bias = (not np.any(inputs["b_attn"])) and \
        (not np.any(inputs["b_proj"]))
    nc = _get_module(zero_bias)
    maps = _in_maps(**inputs)
    res = bass_utils.run_bass_kernel_spmd(
        nc, maps, core_ids=list(range(NCORES)),
        trace=trace, **(trace_kwargs or {}))
    out = np.empty((B, T, C), np.float32)
    for c in range(NCORES):
        b, k = divmod(c, NCORES // B)
        out[b, k * CHUNK:(k + 1) * CHUNK] = np.asarray(
            res.results[c]["y"], dtype=np.float32)
    return out, res


def kernel(x, W_attn, b_attn, W_proj, b_proj):
    inputs = dict(x=np.asarray(x, np.float32), W_attn=W_attn, b_attn=b_attn,
                  W_proj=W_proj, b_proj=b_proj)
    out, _ = _run(inputs)
    return out
